# revision 23
# baseline (speedup 1.0000x reference)
"""GQA causal prefill attention on 8 TRN2 NeuronCores.

Sharding: head-parallel. Core c computes q heads [4c, 4c+4) against kv head c
(n_rep = 4, so the GQA groups align exactly with the shard; no cross-core
communication).

Per-core algorithm (T=2048 tokens, 4 q heads, head_dim 128):
  - Load k, v; build kT (d,s) tiles via PE transpose. v is augmented with a
    ones column -> v_aug (s, 129) in bf16.
  - S^T tiles are packed into 12-tile (3 PSUM bank) units spanning several
    k-tile rows j, so ONE ScalarE exp instruction covers a whole unit. The
    ACT engine is the binding resource (~1.0 ns/col + ~290 ns/inst); packing
    minimizes the per-instruction overhead (~50 exps per core).
  - Causal diagonal masking is done in-place on the bf16 eT buffer by GpSimd
    affine_select (t_local >= s_local), keeping both DVE and ACT off that
    path. q1-3 and v f32->bf16 casts also run on GpSimd.
  - PV with the e^T blocks as the stationary operand and v_aug streaming:
    out_psum (t=128, 129) accumulates over j; column 128 is the softmax
    denominator. Normalize with a per-partition reciprocal multiply (DVE) and
    DMA the (t, d) tile to DRAM.
"""

import sys
import functools

import numpy as np

if "/opt/trn_rl_repo" not in sys.path:
    sys.path.insert(0, "/opt/trn_rl_repo")

T = 2048
H_TOTAL = 32
N_CORES = 8
H = H_TOTAL // N_CORES  # 4 q heads per core
D = 128
P = 128
NT = T // P  # 16 token tiles
SCALE = 0.08838834764831845
UNIT = 12  # tiles per exp unit = 3 PSUM banks

# column offset of s-tile j's slice inside the per-head packed e^T buffer
_EOFF = [0] * (NT + 1)
for _j in range(NT):
    _EOFF[_j + 1] = _EOFF[_j] + (T - P * _j)
E_COLS = _EOFF[NT]  # 17408


def _split_asc(blocks, tail=None):
    """Chop an ascending stream of (j, ilo, ntiles) blocks into units of
    <= UNIT tiles, splitting blocks at tile granularity. `tail` optionally
    forces the sizes of the final units (e.g. [3, 1] for a short drain)."""
    sizes = []
    total = sum(b[2] for b in blocks)
    if tail:
        head = total - sum(tail)
        sizes = [UNIT] * (head // UNIT)
        if head % UNIT:
            sizes.append(head % UNIT)
        sizes += tail
    else:
        sizes = [UNIT] * (total // UNIT)
        if total % UNIT:
            sizes.append(total % UNIT)
    units = []
    cur = []
    cur_n = 0
    si = 0
    for (j, ilo, ln) in blocks:
        a = 0
        while a < ln:
            take = min(sizes[si] - cur_n, ln - a)
            cur.append((j, ilo + a, take))
            cur_n += take
            a += take
            if cur_n == sizes[si]:
                units.append(cur)
                cur = []
                cur_n = 0
                si += 1
    assert not cur
    return units


def _head_units(h):
    """Per head: list of units; unit = list of pieces (j, ilo, ntiles).
    Head 0 starts with descending block-aligned groups (j=15..8) so compute
    begins after only the tail k/q DMA chunks have landed; then j=0..7
    ascending. Heads 1-3 run j ascending. Head 3's final blocks get their own
    shrinking units so the big late PV chains unlock progressively and the
    post-exp drain tail stays short."""
    if h == 0:
        units = [
            [(j, j, NT - j) for j in (15, 14, 13, 12)],  # 10 tiles
            [(j, j, NT - j) for j in (11, 10)],  # 11 tiles
            [(9, 9, 7)],
            [(8, 8, 8)],
            [(7, 7, 9)],
            [(6, 6, 10)],
        ]
        units += _split_asc([(j, j, NT - j) for j in range(6)])
        return units
    tail = [7, 6, 5, 4, 3, 2, 1] if h == H - 1 else None
    return _split_asc([(j, j, NT - j) for j in range(NT)], tail=tail)


def _build_body(tc, nc, q_d, k_d, v_d, o_d, ctx):
    from collections import deque

    import concourse.mybir as mybir
    from concourse.masks import make_identity

    f32 = mybir.dt.float32
    bf16 = mybir.dt.bfloat16

    const = ctx.enter_context(tc.tile_pool(name="const", bufs=1))
    qbp = ctx.enter_context(tc.tile_pool(name="qbf", bufs=4))
    qtp = ctx.enter_context(tc.tile_pool(name="qT", bufs=4))
    ep = ctx.enter_context(tc.tile_pool(name="eT", bufs=2))
    outp = ctx.enter_context(tc.tile_pool(name="outt", bufs=4))
    recp = ctx.enter_context(tc.tile_pool(name="rec", bufs=4))

    # PSUM: two 3-bank S^T units (ping-pong) + two shared 1-bank slots for
    # PV accumulators and transpose staging = exactly 8 banks.
    st_pool = ctx.enter_context(tc.tile_pool(name="st", bufs=2, space="PSUM"))
    sm_pool = ctx.enter_context(tc.tile_pool(name="smp", bufs=2, space="PSUM"))

    # DMA (all on the SP ring, FIFO). k/q0 chunk order matches the compute
    # order: head 0 walks j=15..8 first (needs k/q tail chunks), then 0..7.
    k_view = k_d.rearrange("(j p) d -> p j d", p=P)
    q_view = q_d.rearrange("(i p) h d -> p i h d", p=P)
    o_view = o_d.rearrange("(i p) h d -> p i h d", p=P)
    v_view = v_d.rearrange("(j p) d -> p j d", p=P)

    k_sb = const.tile([P, NT, D], bf16, tag="ksb")
    # all 4 heads staged together: the DRAM-side (h d) line is 1024B
    # contiguous per (i, partition), twice the descriptor payload of a
    # single-head load -> much better DMA efficiency.
    q_all = qbp.tile([P, NT, H, D], bf16, tag="qstg", name="qall", bufs=1)
    v_sb = const.tile([P, NT, D], bf16, tag="vsb")

    def dma_k(b):
        nc.sync.dma_start(k_sb[:, 4 * b:4 * b + 4, :], k_view[:, 4 * b:4 * b + 4, :])

    def dma_q(b):
        nc.sync.dma_start(
            q_all[:, 4 * b:4 * b + 4, :, :], q_view[:, 4 * b:4 * b + 4, :, :])

    dma_k(3); dma_q(3)
    dma_k(2); dma_q(2)
    dma_k(1); dma_q(1)
    dma_k(0); dma_q(0)
    for b in range(2):
        nc.sync.dma_start(v_sb[:, 8 * b:8 * b + 8, :], v_view[:, 8 * b:8 * b + 8, :])

    identity = const.tile([P, P], bf16, tag="ident")
    make_identity(nc, identity)
    zfill = nc.gpsimd.to_reg(0.0)

    # Prewarm the ACT function table so the first real exp doesn't pay the
    # ~1.5us table load on the critical path.
    warm_sb = recp.tile([P, 1], f32, tag="rec", name="warm")
    nc.scalar.activation(
        out=warm_sb, in_=identity[:, 0:1],
        func=mybir.ActivationFunctionType.Exp,
    )

    # Short HAM pre-warm: dummy transposes keep the PE busy during the DMA
    # wait so the clock gate is fully open when real work starts.
    warm_ps = sm_pool.tile([P, 4 * P], bf16, tag="sm", name="warmps")
    for _ in range(10):
        nc.tensor.transpose(warm_ps[0:64, 0:P], identity[:, 0:64], identity)

    def transpose_batch(dst, src, b):
        """Transpose 4 (128,128) bf16 tiles src[:, 4b+m, :] into dst[:, 4b+m, :]
        through one 1-bank PSUM tile and a single batched copy."""
        tp = sm_pool.tile([P, 4 * P], bf16, tag="sm")
        for m in range(4):
            nc.tensor.transpose(
                tp[:, m * P:(m + 1) * P], src[:, 4 * b + m, :], identity)
        nc.vector.tensor_copy(out=dst[:, 4 * b:4 * b + 4, :], in_=tp)

    kT = const.tile([P, NT, P], bf16, tag="kT")  # [d, j, s]
    qT = [
        qtp.tile([P, NT, P], bf16, tag="qT", name=f"qT{h}") for h in range(H)
    ]  # [d, i, t]
    v_aug = const.tile([P, NT, D + 1], bf16, tag="vaug")

    def k_batch(b):
        transpose_batch(kT, k_sb, b)

    def q_tr(h, b):
        """Transpose 4 q tiles of head h from the merged staging buffer."""
        tp = sm_pool.tile([P, 4 * P], bf16, tag="sm")
        for m in range(4):
            nc.tensor.transpose(
                tp[:, m * P:(m + 1) * P], q_all[:, 4 * b + m, h, :], identity)
        nc.vector.tensor_copy(out=qT[h][:, 4 * b:4 * b + 4, :], in_=tp)

    def q0_batch(b):
        q_tr(0, b)

    def v_cast(b):
        nc.vector.tensor_copy(
            out=v_aug[:, 4 * b:4 * b + 4, 0:D], in_=v_sb[:, 4 * b:4 * b + 4, :])

    def v_ones():
        nc.vector.memset(v_aug[:, :, D:D + 1], 1.0)

    # prologue: first k/q0 tail chunks feed head 0's descending start
    k_batch(3)
    q0_batch(3)

    # Static filler schedule: (head, unit_idx) -> list of thunks, emitted
    # just before that unit's S^T matmuls.
    fillers = {}

    def add_f(h, ui, *thunks):
        fillers.setdefault((h, ui), []).extend(thunks)

    add_f(0, 1, lambda: k_batch(2), lambda: q0_batch(2))
    add_f(0, 4, lambda: k_batch(1), lambda: q0_batch(1))
    add_f(0, 6, lambda: k_batch(0), lambda: q0_batch(0))
    add_f(0, 7, lambda: v_cast(0), lambda: v_cast(1), v_ones)
    add_f(0, 8, lambda: v_cast(2), lambda: v_cast(3))
    for b in range(4):
        add_f(0, 9 + b, lambda b=b: q_tr(1, b))
    for hh in (1, 2):
        for b in range(4):
            add_f(hh, 2 + 2 * b, lambda hh=hh, b=b: q_tr(hh + 1, b))

    def emit_chain(eT, h, i):
        """PV accumulation for t-tile i of head h: out_psum (t,129); col 128 is
        the softmax denominator. Normalize and DMA out."""
        pv = sm_pool.tile([P, P + 1], f32, tag="sm")
        for j in range(i + 1):
            c0 = _EOFF[j] + (i - j) * P
            nc.tensor.matmul(
                pv,
                lhsT=eT[:, c0:c0 + P],
                rhs=v_aug[:, j, :],
                start=(j == 0),
                stop=(j == i),
            )
        rec = recp.tile([P, 1], f32, tag="rec")
        nc.vector.reciprocal(rec, pv[:, D:D + 1])
        ot = outp.tile([P, D], f32, tag="outt")
        nc.vector.tensor_scalar_mul(ot, pv[:, 0:D], rec)
        nc.sync.dma_start(o_view[:, i, h, :], ot)

    ready = deque()  # (eT, head, i) PV chains not yet emitted

    def pop_ready(budget, force=False):
        while ready:
            e2, h2, i2 = ready[0]
            size = i2 + 1
            if not force and size > budget and budget < 16:
                break
            ready.popleft()
            emit_chain(e2, h2, i2)
            budget -= size
            if budget <= 0 and not force:
                break

    for h in range(H):
        eT = ep.tile([P, E_COLS], bf16, tag="eT")
        units = _head_units(h)
        for ui, pieces in enumerate(units):
            for f in fillers.get((h, ui), ()):
                f()
            c_lo = min(_EOFF[j] + (ilo - j) * P for (j, ilo, _ln) in pieces)
            n = sum(ln for (_j, _ilo, ln) in pieces)
            stu = st_pool.tile([P, UNIT * P], f32, tag="st")
            for (j, ilo, ln) in pieces:
                pt0 = (_EOFF[j] + (ilo - j) * P - c_lo) // P
                a = 0
                while a < ln:
                    cl = min(4 - (pt0 + a) % 4, ln - a)
                    nc.tensor.matmul(
                        stu[:, (pt0 + a) * P:(pt0 + a + cl) * P],
                        lhsT=kT[:, j, :],
                        rhs=qT[h][:, ilo + a:ilo + a + cl, :],
                        start=True,
                        stop=True,
                    )
                    a += cl
            nc.scalar.activation(
                out=eT[:, c_lo:c_lo + n * P],
                in_=stu[:, 0:n * P],
                func=mybir.ActivationFunctionType.Exp,
                scale=SCALE,
            )
            for (j, ilo, _ln) in pieces:
                if ilo == j:
                    # causal mask on the diagonal tile: keep t_local >= s_local
                    nc.gpsimd.affine_select(
                        out=eT[:, _EOFF[j]:_EOFF[j] + P],
                        in_=eT[:, _EOFF[j]:_EOFF[j] + P],
                        pattern=[[1, P]],
                        compare_op=mybir.AluOpType.is_ge,
                        fill=zfill,
                        base=0,
                        channel_multiplier=-1,
                    )
                    if h > 0 or j < 6:
                        ready.append((eT, h, j))
            budget = UNIT
            if ready and ready[0][1] < h:
                budget += 6
            if h == H - 1:
                budget += 8
            pop_ready(budget)
        if h == 0:
            for i in range(6, NT):
                ready.append((eT, 0, i))
        if h >= 2:
            # chains two heads back must drain before their eT slot recycles
            while ready and ready[0][1] < h - 1:
                e2, h2, i2 = ready.popleft()
                emit_chain(e2, h2, i2)
    pop_ready(0, force=True)


@functools.lru_cache(maxsize=1)
def _build():
    import concourse.tile as tile
    import concourse.mybir as mybir
    from concourse import bacc
    from contextlib import ExitStack

    f32 = mybir.dt.float32
    bf16 = mybir.dt.bfloat16
    nc = bacc.Bacc(
        "TRN2",
        target_bir_lowering=False,
        debug=False,
        num_devices=N_CORES,
    )
    # q/k/v are pre-cast to bf16 on the host (the kernel computes in bf16
    # anyway), halving input DMA bytes.
    q_d = nc.dram_tensor("q", (T, H, D), bf16, kind="ExternalInput").ap()
    k_d = nc.dram_tensor("k", (T, D), bf16, kind="ExternalInput").ap()
    v_d = nc.dram_tensor("v", (T, D), bf16, kind="ExternalInput").ap()
    o_d = nc.dram_tensor("out", (T, H, D), f32, kind="ExternalOutput").ap()

    with tile.TileContext(nc) as tc:
        with ExitStack() as ctx:
            _build_body(tc, nc, q_d, k_d, v_d, o_d, ctx)
    nc.compile()
    return nc


def _in_maps(q, k, v):
    import ml_dtypes

    bf16 = ml_dtypes.bfloat16
    q = np.asarray(q).astype(bf16)
    k = np.asarray(k).astype(bf16)
    v = np.asarray(v).astype(bf16)
    return [
        {
            "q": np.ascontiguousarray(q[:, H * c:H * c + H, :]),
            "k": np.ascontiguousarray(k[:, c, :]),
            "v": np.ascontiguousarray(v[:, c, :]),
        }
        for c in range(N_CORES)
    ]


def kernel(q, k, v, _trace=False):
    from concourse.bass_utils import run_bass_kernel_spmd

    nc = _build()
    res = run_bass_kernel_spmd(
        nc, _in_maps(q, k, v), core_ids=list(range(N_CORES)), trace=_trace
    )
    out = np.empty((T, H_TOTAL, D), dtype=np.float32)
    for c in range(N_CORES):
        out[:, H * c:H * c + H, :] = res.results[c]["out"].reshape(T, H, D)
    if _trace:
        return out, res
    return out


# revision 26
# speedup vs baseline: 1.1712x; 1.1712x over previous
"""GQA causal prefill attention on 8 TRN2 NeuronCores.

Sharding: head-parallel. Core c computes q heads [4c, 4c+4) against kv head c
(n_rep = 4, so the GQA groups align exactly with the shard; no cross-core
communication).

Per-core algorithm (T=2048 tokens, 4 q heads, head_dim 128):
  - Load k, v; build kT (d,s) tiles via PE transpose. v is augmented with a
    ones column -> v_aug (s, 129) in bf16.
  - S^T tiles are packed into 12-tile (3 PSUM bank) units spanning several
    k-tile rows j, so ONE ScalarE exp instruction covers a whole unit. The
    ACT engine is the binding resource (~1.0 ns/col + ~290 ns/inst); packing
    minimizes the per-instruction overhead (~50 exps per core).
  - Causal diagonal masking is done in-place on the bf16 eT buffer by GpSimd
    affine_select (t_local >= s_local), keeping both DVE and ACT off that
    path. q1-3 and v f32->bf16 casts also run on GpSimd.
  - PV with the e^T blocks as the stationary operand and v_aug streaming:
    out_psum (t=128, 129) accumulates over j; column 128 is the softmax
    denominator. Normalize with a per-partition reciprocal multiply (DVE) and
    DMA the (t, d) tile to DRAM.
"""

import sys
import functools

import numpy as np

if "/opt/trn_rl_repo" not in sys.path:
    sys.path.insert(0, "/opt/trn_rl_repo")

T = 2048
H_TOTAL = 32
N_CORES = 8
H = H_TOTAL // N_CORES  # 4 q heads per core
D = 128
P = 128
NT = T // P  # 16 token tiles
SCALE = 0.08838834764831845
UNIT = 12  # tiles per exp unit = 3 PSUM banks

# column offset of s-tile j's slice inside the per-head packed e^T buffer
_EOFF = [0] * (NT + 1)
for _j in range(NT):
    _EOFF[_j + 1] = _EOFF[_j] + (T - P * _j)
E_COLS = _EOFF[NT]  # 17408


def _split_asc(blocks, tail=None):
    """Chop an ascending stream of (j, ilo, ntiles) blocks into units of
    <= UNIT tiles, splitting blocks at tile granularity. `tail` optionally
    forces the sizes of the final units (e.g. [3, 1] for a short drain)."""
    sizes = []
    total = sum(b[2] for b in blocks)
    if tail:
        head = total - sum(tail)
        sizes = [UNIT] * (head // UNIT)
        if head % UNIT:
            sizes.append(head % UNIT)
        sizes += tail
    else:
        sizes = [UNIT] * (total // UNIT)
        if total % UNIT:
            sizes.append(total % UNIT)
    units = []
    cur = []
    cur_n = 0
    si = 0
    for (j, ilo, ln) in blocks:
        a = 0
        while a < ln:
            take = min(sizes[si] - cur_n, ln - a)
            cur.append((j, ilo + a, take))
            cur_n += take
            a += take
            if cur_n == sizes[si]:
                units.append(cur)
                cur = []
                cur_n = 0
                si += 1
    assert not cur
    return units


def _head_units(h):
    """Per head: list of units; unit = list of pieces (j, ilo, ntiles).
    Head 0 starts with descending block-aligned groups (j=15..8) so compute
    begins after only the tail k/q DMA chunks have landed; then j=0..7
    ascending. Heads 1-3 run j ascending. Head 3's final blocks get their own
    shrinking units so the big late PV chains unlock progressively and the
    post-exp drain tail stays short."""
    if h == 0:
        units = [
            [(j, j, NT - j) for j in (15, 14, 13, 12)],  # 10 tiles
            [(j, j, NT - j) for j in (11, 10)],  # 11 tiles
            [(9, 9, 7)],
            [(8, 8, 8)],
            [(7, 7, 9)],
            [(6, 6, 10)],
        ]
        units += _split_asc([(j, j, NT - j) for j in range(6)])
        return units
    tail = [7, 6, 5, 4, 3, 2, 1] if h == H - 1 else None
    return _split_asc([(j, j, NT - j) for j in range(NT)], tail=tail)


def _build_body(tc, nc, q_d, k_d, v_d, o_d, ctx):
    from collections import deque

    import concourse.mybir as mybir
    from concourse.masks import make_identity

    f32 = mybir.dt.float32
    bf16 = mybir.dt.bfloat16

    const = ctx.enter_context(tc.tile_pool(name="const", bufs=1))
    qbp = ctx.enter_context(tc.tile_pool(name="qbf", bufs=4))
    qtp = ctx.enter_context(tc.tile_pool(name="qT", bufs=4))
    ep = ctx.enter_context(tc.tile_pool(name="eT", bufs=2))
    outp = ctx.enter_context(tc.tile_pool(name="outt", bufs=4))
    recp = ctx.enter_context(tc.tile_pool(name="rec", bufs=4))

    # PSUM: two 3-bank S^T units (ping-pong) + two shared 1-bank slots for
    # PV accumulators and transpose staging = exactly 8 banks.
    st_pool = ctx.enter_context(tc.tile_pool(name="st", bufs=2, space="PSUM"))
    sm_pool = ctx.enter_context(tc.tile_pool(name="smp", bufs=2, space="PSUM"))

    # DMA (all on the SP ring, FIFO). k/q0 chunk order matches the compute
    # order: head 0 walks j=15..8 first (needs k/q tail chunks), then 0..7.
    k_view = k_d.rearrange("(j p) d -> p j d", p=P)
    q_view = q_d.rearrange("(i p) h d -> p i h d", p=P)
    o_view = o_d.rearrange("(i p) h d -> p i h d", p=P)
    v_view = v_d.rearrange("(j p) d -> p j d", p=P)

    k_sb = const.tile([P, NT, D], bf16, tag="ksb")
    # all 4 heads staged together: the DRAM-side (h d) line is 1024B
    # contiguous per (i, partition), twice the descriptor payload of a
    # single-head load -> much better DMA efficiency.
    q_all = qbp.tile([P, NT, H, D], bf16, tag="qstg", name="qall", bufs=1)
    v_sb = const.tile([P, NT, D], bf16, tag="vsb")

    def dma_k(b):
        nc.sync.dma_start(k_sb[:, 4 * b:4 * b + 4, :], k_view[:, 4 * b:4 * b + 4, :])

    def dma_q(b):
        nc.sync.dma_start(
            q_all[:, 4 * b:4 * b + 4, :, :], q_view[:, 4 * b:4 * b + 4, :, :])

    def dma_q_h0(b):
        nc.sync.dma_start(
            q_all[:, 4 * b:4 * b + 4, 0, :], q_view[:, 4 * b:4 * b + 4, 0, :])

    def dma_q_h123(b):
        nc.sync.dma_start(
            q_all[:, 4 * b:4 * b + 4, 1:H, :], q_view[:, 4 * b:4 * b + 4, 1:H, :])

    # Head 0's descending start needs only its own tail q tiles; split those
    # two chunks so the critical 128KB lands before the other heads' 384KB.
    dma_k(3); dma_q_h0(3)
    dma_k(2); dma_q_h0(2)
    dma_q_h123(3)
    dma_k(1); dma_q(1)
    dma_q_h123(2)
    dma_k(0); dma_q(0)
    for b in range(2):
        nc.sync.dma_start(v_sb[:, 8 * b:8 * b + 8, :], v_view[:, 8 * b:8 * b + 8, :])

    identity = const.tile([P, P], bf16, tag="ident")
    make_identity(nc, identity)
    zfill = nc.gpsimd.to_reg(0.0)

    # Prewarm the ACT function table so the first real exp doesn't pay the
    # ~1.5us table load on the critical path.
    warm_sb = recp.tile([P, 1], f32, tag="rec", name="warm")
    nc.scalar.activation(
        out=warm_sb, in_=identity[:, 0:1],
        func=mybir.ActivationFunctionType.Exp,
    )

    # Short HAM pre-warm: dummy transposes keep the PE busy during the DMA
    # wait so the clock gate is fully open when real work starts.
    warm_ps = sm_pool.tile([P, 4 * P], bf16, tag="sm", name="warmps")
    for _ in range(10):
        nc.tensor.transpose(warm_ps[0:64, 0:P], identity[:, 0:64], identity)

    def transpose_batch(dst, src, b):
        """Transpose 4 (128,128) bf16 tiles src[:, 4b+m, :] into dst[:, 4b+m, :]
        through one 1-bank PSUM tile and a single batched copy."""
        tp = sm_pool.tile([P, 4 * P], bf16, tag="sm")
        for m in range(4):
            nc.tensor.transpose(
                tp[:, m * P:(m + 1) * P], src[:, 4 * b + m, :], identity)
        nc.vector.tensor_copy(out=dst[:, 4 * b:4 * b + 4, :], in_=tp)

    kT = const.tile([P, NT, P], bf16, tag="kT")  # [d, j, s]
    qT = [
        qtp.tile([P, NT, P], bf16, tag="qT", name=f"qT{h}") for h in range(H)
    ]  # [d, i, t]
    v_aug = const.tile([P, NT, D + 1], bf16, tag="vaug")

    def k_batch(b):
        transpose_batch(kT, k_sb, b)

    def q_tr(h, b):
        """Transpose 4 q tiles of head h from the merged staging buffer."""
        tp = sm_pool.tile([P, 4 * P], bf16, tag="sm")
        for m in range(4):
            nc.tensor.transpose(
                tp[:, m * P:(m + 1) * P], q_all[:, 4 * b + m, h, :], identity)
        nc.vector.tensor_copy(out=qT[h][:, 4 * b:4 * b + 4, :], in_=tp)

    def q0_batch(b):
        q_tr(0, b)

    def v_cast(b):
        nc.vector.tensor_copy(
            out=v_aug[:, 4 * b:4 * b + 4, 0:D], in_=v_sb[:, 4 * b:4 * b + 4, :])

    def v_ones():
        nc.vector.memset(v_aug[:, :, D:D + 1], 1.0)

    # prologue: first k/q0 tail chunks feed head 0's descending start
    k_batch(3)
    q0_batch(3)

    # Static filler schedule: (head, unit_idx) -> list of thunks, emitted
    # just before that unit's S^T matmuls.
    fillers = {}

    def add_f(h, ui, *thunks):
        fillers.setdefault((h, ui), []).extend(thunks)

    add_f(0, 1, lambda: k_batch(2), lambda: q0_batch(2))
    add_f(0, 4, lambda: k_batch(1), lambda: q0_batch(1))
    add_f(0, 6, lambda: k_batch(0), lambda: q0_batch(0))
    add_f(0, 7, lambda: v_cast(0), lambda: v_cast(1), v_ones)
    add_f(0, 8, lambda: v_cast(2), lambda: v_cast(3))
    for b in range(4):
        add_f(0, 9 + b, lambda b=b: q_tr(1, b))
    for hh in (1, 2):
        for b in range(4):
            add_f(hh, 2 + 2 * b, lambda hh=hh, b=b: q_tr(hh + 1, b))

    def pv_matmuls(pv, eT, i, j0, j1):
        for j in range(j0, j1 + 1):
            c0 = _EOFF[j] + (i - j) * P
            nc.tensor.matmul(
                pv,
                lhsT=eT[:, c0:c0 + P],
                rhs=v_aug[:, j, :],
                start=(j == j0),
                stop=(j == j1),
            )

    def normalize_out(tot, h, i):
        rec = recp.tile([P, 1], f32, tag="rec")
        nc.vector.reciprocal(rec, tot[:, D:D + 1])
        ot = outp.tile([P, D], f32, tag="outt")
        nc.vector.tensor_scalar_mul(ot, tot[:, 0:D], rec)
        nc.sync.dma_start(o_view[:, i, h, :], ot)

    def emit_chain(eT, h, i):
        """PV accumulation for t-tile i of head h: out_psum (t,129); col 128 is
        the softmax denominator. Normalize and DMA out."""
        pv = sm_pool.tile([P, P + 1], f32, tag="sm")
        pv_matmuls(pv, eT, i, 0, i)
        normalize_out(pv, h, i)

    # Two-segment chains for the last head's late t-tiles: the j<=PSPLIT
    # prefix runs as soon as those blocks are exp'd (before the final tiny
    # units), gets spilled to SBUF, and only the short suffix + a DVE merge
    # remain after block i's exp -> the post-last-exp drain stays ~1 chain.
    PSPLIT = 8
    part1_sb = {}

    def emit_part1(eT, i):
        pv = sm_pool.tile([P, P + 1], f32, tag="sm")
        pv_matmuls(pv, eT, i, 0, PSPLIT)
        stg = recp.tile([P, P + 1], f32, tag="p1", bufs=7, name=f"p1_{i}")
        nc.vector.tensor_copy(out=stg, in_=pv)
        part1_sb[i] = stg

    def emit_finalize(eT, h, i):
        stg = part1_sb.pop(i)
        pv = sm_pool.tile([P, P + 1], f32, tag="sm")
        pv_matmuls(pv, eT, i, PSPLIT + 1, i)
        tot = recp.tile([P, P + 1], f32, tag="ptot", bufs=2, name=f"ptot{i}")
        nc.vector.tensor_tensor(tot, pv, stg, mybir.AluOpType.add)
        normalize_out(tot, h, i)

    ready = deque()  # (eT, head, i) PV chains not yet emitted

    def pop_ready(budget, force=False):
        while ready:
            e2, h2, i2 = ready[0]
            size = i2 + 1
            if not force and size > budget and budget < 16:
                break
            ready.popleft()
            emit_chain(e2, h2, i2)
            budget -= size
            if budget <= 0 and not force:
                break

    for h in range(H):
        eT = ep.tile([P, E_COLS], bf16, tag="eT")
        units = _head_units(h)
        for ui, pieces in enumerate(units):
            for f in fillers.get((h, ui), ()):
                f()
            c_lo = min(_EOFF[j] + (ilo - j) * P for (j, ilo, _ln) in pieces)
            n = sum(ln for (_j, _ilo, ln) in pieces)
            stu = st_pool.tile([P, UNIT * P], f32, tag="st")
            for (j, ilo, ln) in pieces:
                pt0 = (_EOFF[j] + (ilo - j) * P - c_lo) // P
                a = 0
                while a < ln:
                    cl = min(4 - (pt0 + a) % 4, ln - a)
                    nc.tensor.matmul(
                        stu[:, (pt0 + a) * P:(pt0 + a + cl) * P],
                        lhsT=kT[:, j, :],
                        rhs=qT[h][:, ilo + a:ilo + a + cl, :],
                        start=True,
                        stop=True,
                    )
                    a += cl
            nc.scalar.activation(
                out=eT[:, c_lo:c_lo + n * P],
                in_=stu[:, 0:n * P],
                func=mybir.ActivationFunctionType.Exp,
                scale=SCALE,
            )
            late3 = h == H - 1 and ui >= 8
            for (j, ilo, _ln) in pieces:
                if ilo == j:
                    # causal mask on the diagonal tile: keep t_local >= s_local
                    nc.gpsimd.affine_select(
                        out=eT[:, _EOFF[j]:_EOFF[j] + P],
                        in_=eT[:, _EOFF[j]:_EOFF[j] + P],
                        pattern=[[1, P]],
                        compare_op=mybir.AluOpType.is_ge,
                        fill=zfill,
                        base=0,
                        channel_multiplier=-1,
                    )
                    if late3 and j > PSPLIT:
                        emit_finalize(eT, h, j)
                    elif h > 0 or j < 6:
                        ready.append((eT, h, j))
            budget = UNIT
            if ready and ready[0][1] < h:
                budget += 6
            if h == H - 1:
                budget += 8
            pop_ready(budget)
            if late3:
                for i in {8: (9, 10), 9: (11, 12), 10: (13, 14), 11: (15,)}.get(
                        ui, ()):
                    emit_part1(eT, i)
        if h == 0:
            for i in range(6, NT):
                ready.append((eT, 0, i))
        if h >= 2:
            # chains two heads back must drain before their eT slot recycles
            while ready and ready[0][1] < h - 1:
                e2, h2, i2 = ready.popleft()
                emit_chain(e2, h2, i2)
    pop_ready(0, force=True)


@functools.lru_cache(maxsize=1)
def _build():
    import concourse.tile as tile
    import concourse.mybir as mybir
    from concourse import bacc
    from contextlib import ExitStack

    f32 = mybir.dt.float32
    bf16 = mybir.dt.bfloat16
    nc = bacc.Bacc(
        "TRN2",
        target_bir_lowering=False,
        debug=False,
        num_devices=N_CORES,
    )
    # q/k/v are pre-cast to bf16 on the host (the kernel computes in bf16
    # anyway), halving input DMA bytes.
    q_d = nc.dram_tensor("q", (T, H, D), bf16, kind="ExternalInput").ap()
    k_d = nc.dram_tensor("k", (T, D), bf16, kind="ExternalInput").ap()
    v_d = nc.dram_tensor("v", (T, D), bf16, kind="ExternalInput").ap()
    o_d = nc.dram_tensor("out", (T, H, D), f32, kind="ExternalOutput").ap()

    with tile.TileContext(nc) as tc:
        with ExitStack() as ctx:
            _build_body(tc, nc, q_d, k_d, v_d, o_d, ctx)
    nc.compile()
    return nc


def _in_maps(q, k, v):
    import ml_dtypes

    bf16 = ml_dtypes.bfloat16
    q = np.asarray(q).astype(bf16)
    k = np.asarray(k).astype(bf16)
    v = np.asarray(v).astype(bf16)
    return [
        {
            "q": np.ascontiguousarray(q[:, H * c:H * c + H, :]),
            "k": np.ascontiguousarray(k[:, c, :]),
            "v": np.ascontiguousarray(v[:, c, :]),
        }
        for c in range(N_CORES)
    ]


def kernel(q, k, v, _trace=False):
    from concourse.bass_utils import run_bass_kernel_spmd

    nc = _build()
    res = run_bass_kernel_spmd(
        nc, _in_maps(q, k, v), core_ids=list(range(N_CORES)), trace=_trace
    )
    out = np.empty((T, H_TOTAL, D), dtype=np.float32)
    for c in range(N_CORES):
        out[:, H * c:H * c + H, :] = res.results[c]["out"].reshape(T, H, D)
    if _trace:
        return out, res
    return out


# revision 29
# speedup vs baseline: 1.1792x; 1.0068x over previous
"""GQA causal prefill attention on 8 TRN2 NeuronCores.

Sharding: head-parallel. Core c computes q heads [4c, 4c+4) against kv head c
(n_rep = 4, so the GQA groups align exactly with the shard; no cross-core
communication).

Per-core algorithm (T=2048 tokens, 4 q heads, head_dim 128):
  - Load k, v; build kT (d,s) tiles via PE transpose. v is augmented with a
    ones column -> v_aug (s, 129) in bf16.
  - S^T tiles are packed into 12-tile (3 PSUM bank) units spanning several
    k-tile rows j, so ONE ScalarE exp instruction covers a whole unit. The
    ACT engine is the binding resource (~1.0 ns/col + ~290 ns/inst); packing
    minimizes the per-instruction overhead (~50 exps per core).
  - Causal diagonal masking is done in-place on the bf16 eT buffer by GpSimd
    affine_select (t_local >= s_local), keeping both DVE and ACT off that
    path. q1-3 and v f32->bf16 casts also run on GpSimd.
  - PV with the e^T blocks as the stationary operand and v_aug streaming:
    out_psum (t=128, 129) accumulates over j; column 128 is the softmax
    denominator. Normalize with a per-partition reciprocal multiply (DVE) and
    DMA the (t, d) tile to DRAM.
"""

import sys
import functools

import numpy as np

if "/opt/trn_rl_repo" not in sys.path:
    sys.path.insert(0, "/opt/trn_rl_repo")

T = 2048
H_TOTAL = 32
N_CORES = 8
H = H_TOTAL // N_CORES  # 4 q heads per core
D = 128
P = 128
NT = T // P  # 16 token tiles
SCALE = 0.08838834764831845
UNIT = 12  # tiles per exp unit = 3 PSUM banks

# column offset of s-tile j's slice inside the per-head packed e^T buffer
_EOFF = [0] * (NT + 1)
for _j in range(NT):
    _EOFF[_j + 1] = _EOFF[_j] + (T - P * _j)
E_COLS = _EOFF[NT]  # 17408


def _split_asc(blocks, tail=None):
    """Chop an ascending stream of (j, ilo, ntiles) blocks into units of
    <= UNIT tiles, splitting blocks at tile granularity. `tail` optionally
    forces the sizes of the final units (e.g. [3, 1] for a short drain)."""
    sizes = []
    total = sum(b[2] for b in blocks)
    if tail:
        head = total - sum(tail)
        sizes = [UNIT] * (head // UNIT)
        if head % UNIT:
            sizes.append(head % UNIT)
        sizes += tail
    else:
        sizes = [UNIT] * (total // UNIT)
        if total % UNIT:
            sizes.append(total % UNIT)
    units = []
    cur = []
    cur_n = 0
    si = 0
    for (j, ilo, ln) in blocks:
        a = 0
        while a < ln:
            take = min(sizes[si] - cur_n, ln - a)
            cur.append((j, ilo + a, take))
            cur_n += take
            a += take
            if cur_n == sizes[si]:
                units.append(cur)
                cur = []
                cur_n = 0
                si += 1
    assert not cur
    return units


def _head_units(h):
    """Per head: list of units; unit = list of pieces (j, ilo, ntiles).
    Head 0 starts with descending block-aligned groups (j=15..8) so compute
    begins after only the tail k/q DMA chunks have landed; then j=0..7
    ascending. Heads 1-3 run j ascending. Head 3's final blocks get their own
    shrinking units so the big late PV chains unlock progressively and the
    post-exp drain tail stays short."""
    if h == 0:
        units = [
            [(15, 15, 1), (14, 14, 2)],  # 3 tiles: first exp ASAP
            [(13, 13, 3), (12, 12, 4)],  # 7 tiles
            [(j, j, NT - j) for j in (11, 10)],  # 11 tiles
            [(9, 9, 7)],
            [(8, 8, 8)],
            [(7, 7, 9)],
            [(6, 6, 10)],
        ]
        units += _split_asc([(j, j, NT - j) for j in range(6)])
        return units
    tail = [7, 6, 5, 4, 3, 2, 1] if h == H - 1 else None
    return _split_asc([(j, j, NT - j) for j in range(NT)], tail=tail)


def _build_body(tc, nc, q_d, k_d, v_d, o_d, ctx):
    from collections import deque

    import concourse.mybir as mybir
    from concourse.masks import make_identity

    f32 = mybir.dt.float32
    bf16 = mybir.dt.bfloat16

    const = ctx.enter_context(tc.tile_pool(name="const", bufs=1))
    qbp = ctx.enter_context(tc.tile_pool(name="qbf", bufs=4))
    qtp = ctx.enter_context(tc.tile_pool(name="qT", bufs=4))
    ep = ctx.enter_context(tc.tile_pool(name="eT", bufs=2))
    outp = ctx.enter_context(tc.tile_pool(name="outt", bufs=4))
    recp = ctx.enter_context(tc.tile_pool(name="rec", bufs=4))

    # PSUM: two 3-bank S^T units (ping-pong) + two shared 1-bank slots for
    # PV accumulators and transpose staging = exactly 8 banks.
    st_pool = ctx.enter_context(tc.tile_pool(name="st", bufs=2, space="PSUM"))
    sm_pool = ctx.enter_context(tc.tile_pool(name="smp", bufs=2, space="PSUM"))

    # DMA (all on the SP ring, FIFO). k/q0 chunk order matches the compute
    # order: head 0 walks j=15..8 first (needs k/q tail chunks), then 0..7.
    k_view = k_d.rearrange("(j p) d -> p j d", p=P)
    q_view = q_d.rearrange("(i p) h d -> p i h d", p=P)
    o_view = o_d.rearrange("(i p) h d -> p i h d", p=P)
    v_view = v_d.rearrange("(j p) d -> p j d", p=P)

    k_sb = const.tile([P, NT, D], bf16, tag="ksb")
    # all 4 heads staged together: the DRAM-side (h d) line is 1024B
    # contiguous per (i, partition), twice the descriptor payload of a
    # single-head load -> much better DMA efficiency.
    q_all = qbp.tile([P, NT, H, D], bf16, tag="qstg", name="qall", bufs=1)
    v_sb = const.tile([P, NT, D], bf16, tag="vsb")

    def dma_k(b):
        nc.sync.dma_start(k_sb[:, 4 * b:4 * b + 4, :], k_view[:, 4 * b:4 * b + 4, :])

    def dma_q(b):
        nc.sync.dma_start(
            q_all[:, 4 * b:4 * b + 4, :, :], q_view[:, 4 * b:4 * b + 4, :, :])

    def dma_q_h0(b):
        nc.sync.dma_start(
            q_all[:, 4 * b:4 * b + 4, 0, :], q_view[:, 4 * b:4 * b + 4, 0, :])

    def dma_q_h123(b):
        nc.sync.dma_start(
            q_all[:, 4 * b:4 * b + 4, 1:H, :], q_view[:, 4 * b:4 * b + 4, 1:H, :])

    # Everything head 0 needs lands first (its j walk is 15..8 then 0..7, so
    # k/q tail chunks lead); v next (first PV chains start ~20us in); the
    # other heads' q last (not needed until their transpose fillers).
    dma_k(3); dma_q_h0(3)
    dma_k(2); dma_q_h0(2)
    dma_k(1); dma_q_h0(1)
    dma_k(0); dma_q_h0(0)
    for b in range(2):
        nc.sync.dma_start(v_sb[:, 8 * b:8 * b + 8, :], v_view[:, 8 * b:8 * b + 8, :])
    for b in (3, 2, 1, 0):
        dma_q_h123(b)

    identity = const.tile([P, P], bf16, tag="ident")
    make_identity(nc, identity)
    zfill = nc.gpsimd.to_reg(0.0)

    # Prewarm the ACT function table so the first real exp doesn't pay the
    # ~1.5us table load on the critical path.
    warm_sb = recp.tile([P, 1], f32, tag="rec", name="warm")
    nc.scalar.activation(
        out=warm_sb, in_=identity[:, 0:1],
        func=mybir.ActivationFunctionType.Exp,
    )

    # Short HAM pre-warm: dummy transposes keep the PE busy during the DMA
    # wait so the clock gate is fully open when real work starts.
    warm_ps = sm_pool.tile([P, 4 * P], bf16, tag="sm", name="warmps")
    for _ in range(10):
        nc.tensor.transpose(warm_ps[0:64, 0:P], identity[:, 0:64], identity)

    def transpose_batch(dst, src, b):
        """Transpose 4 (128,128) bf16 tiles src[:, 4b+m, :] into dst[:, 4b+m, :]
        through one 1-bank PSUM tile and a single batched copy."""
        tp = sm_pool.tile([P, 4 * P], bf16, tag="sm")
        for m in range(4):
            nc.tensor.transpose(
                tp[:, m * P:(m + 1) * P], src[:, 4 * b + m, :], identity)
        nc.vector.tensor_copy(out=dst[:, 4 * b:4 * b + 4, :], in_=tp)

    kT = const.tile([P, NT, P], bf16, tag="kT")  # [d, j, s]
    qT = [
        qtp.tile([P, NT, P], bf16, tag="qT", name=f"qT{h}") for h in range(H)
    ]  # [d, i, t]
    v_aug = const.tile([P, NT, D + 1], bf16, tag="vaug")

    def k_batch(b):
        transpose_batch(kT, k_sb, b)

    def q_tr(h, b):
        """Transpose 4 q tiles of head h from the merged staging buffer."""
        tp = sm_pool.tile([P, 4 * P], bf16, tag="sm")
        for m in range(4):
            nc.tensor.transpose(
                tp[:, m * P:(m + 1) * P], q_all[:, 4 * b + m, h, :], identity)
        nc.vector.tensor_copy(out=qT[h][:, 4 * b:4 * b + 4, :], in_=tp)

    def q0_batch(b):
        q_tr(0, b)

    def v_cast(b):
        nc.vector.tensor_copy(
            out=v_aug[:, 4 * b:4 * b + 4, 0:D], in_=v_sb[:, 4 * b:4 * b + 4, :])

    def v_ones():
        nc.vector.memset(v_aug[:, :, D:D + 1], 1.0)

    # prologue: first k/q0 tail chunks feed head 0's descending start
    k_batch(3)
    q0_batch(3)

    # Static filler schedule: (head, unit_idx) -> list of thunks, emitted
    # just before that unit's S^T matmuls.
    fillers = {}

    def add_f(h, ui, *thunks):
        fillers.setdefault((h, ui), []).extend(thunks)

    add_f(0, 2, lambda: k_batch(2), lambda: q0_batch(2))
    add_f(0, 5, lambda: k_batch(1), lambda: q0_batch(1))
    add_f(0, 7, lambda: k_batch(0), lambda: q0_batch(0))
    add_f(0, 8, lambda: v_cast(0), lambda: v_cast(1), v_ones)
    add_f(0, 9, lambda: v_cast(2), lambda: v_cast(3))
    for b in range(4):
        add_f(0, 10 + b, lambda b=b: q_tr(1, b))
    for hh in (1, 2):
        for b in range(4):
            add_f(hh, 2 + 2 * b, lambda hh=hh, b=b: q_tr(hh + 1, b))

    def pv_matmuls(pv, eT, i, j0, j1):
        for j in range(j0, j1 + 1):
            c0 = _EOFF[j] + (i - j) * P
            nc.tensor.matmul(
                pv,
                lhsT=eT[:, c0:c0 + P],
                rhs=v_aug[:, j, :],
                start=(j == j0),
                stop=(j == j1),
            )

    def normalize_out(tot, h, i):
        rec = recp.tile([P, 1], f32, tag="rec")
        nc.vector.reciprocal(rec, tot[:, D:D + 1])
        ot = outp.tile([P, D], f32, tag="outt")
        nc.vector.tensor_scalar_mul(ot, tot[:, 0:D], rec)
        nc.sync.dma_start(o_view[:, i, h, :], ot)

    def emit_chain(eT, h, i):
        """PV accumulation for t-tile i of head h: out_psum (t,129); col 128 is
        the softmax denominator. Normalize and DMA out."""
        pv = sm_pool.tile([P, P + 1], f32, tag="sm")
        pv_matmuls(pv, eT, i, 0, i)
        normalize_out(pv, h, i)

    # Two-segment chains for the last head's late t-tiles: the j<=PSPLIT
    # prefix runs as soon as those blocks are exp'd (before the final tiny
    # units), gets spilled to SBUF, and only the short suffix + a DVE merge
    # remain after block i's exp -> the post-last-exp drain stays ~1 chain.
    PSPLIT = 8
    part1_sb = {}

    def emit_part1(eT, i):
        pv = sm_pool.tile([P, P + 1], f32, tag="sm")
        pv_matmuls(pv, eT, i, 0, PSPLIT)
        stg = recp.tile([P, P + 1], f32, tag="p1", bufs=7, name=f"p1_{i}")
        nc.vector.tensor_copy(out=stg, in_=pv)
        part1_sb[i] = stg

    def emit_finalize(eT, h, i):
        stg = part1_sb.pop(i)
        pv = sm_pool.tile([P, P + 1], f32, tag="sm")
        pv_matmuls(pv, eT, i, PSPLIT + 1, i)
        tot = recp.tile([P, P + 1], f32, tag="ptot", bufs=2, name=f"ptot{i}")
        nc.vector.tensor_tensor(tot, pv, stg, mybir.AluOpType.add)
        normalize_out(tot, h, i)

    ready = deque()  # (eT, head, i) PV chains not yet emitted

    def pop_ready(budget, force=False):
        while ready:
            e2, h2, i2 = ready[0]
            size = i2 + 1
            if not force and size > budget and budget < 16:
                break
            ready.popleft()
            emit_chain(e2, h2, i2)
            budget -= size
            if budget <= 0 and not force:
                break

    for h in range(H):
        eT = ep.tile([P, E_COLS], bf16, tag="eT")
        units = _head_units(h)
        for ui, pieces in enumerate(units):
            for f in fillers.get((h, ui), ()):
                f()
            c_lo = min(_EOFF[j] + (ilo - j) * P for (j, ilo, _ln) in pieces)
            n = sum(ln for (_j, _ilo, ln) in pieces)
            stu = st_pool.tile([P, UNIT * P], f32, tag="st")
            for (j, ilo, ln) in pieces:
                pt0 = (_EOFF[j] + (ilo - j) * P - c_lo) // P
                a = 0
                while a < ln:
                    cl = min(4 - (pt0 + a) % 4, ln - a)
                    nc.tensor.matmul(
                        stu[:, (pt0 + a) * P:(pt0 + a + cl) * P],
                        lhsT=kT[:, j, :],
                        rhs=qT[h][:, ilo + a:ilo + a + cl, :],
                        start=True,
                        stop=True,
                    )
                    a += cl
            nc.scalar.activation(
                out=eT[:, c_lo:c_lo + n * P],
                in_=stu[:, 0:n * P],
                func=mybir.ActivationFunctionType.Exp,
                scale=SCALE,
            )
            late3 = h == H - 1 and ui >= 8
            for (j, ilo, _ln) in pieces:
                if ilo == j:
                    # causal mask on the diagonal tile: keep t_local >= s_local
                    nc.gpsimd.affine_select(
                        out=eT[:, _EOFF[j]:_EOFF[j] + P],
                        in_=eT[:, _EOFF[j]:_EOFF[j] + P],
                        pattern=[[1, P]],
                        compare_op=mybir.AluOpType.is_ge,
                        fill=zfill,
                        base=0,
                        channel_multiplier=-1,
                    )
                    if late3 and j > PSPLIT:
                        emit_finalize(eT, h, j)
                    elif h > 0 or j < 6:
                        ready.append((eT, h, j))
            budget = UNIT
            if ready and ready[0][1] < h:
                budget += 6
            if h == H - 1:
                budget += 8
            pop_ready(budget)
            if late3:
                for i in {8: (9, 10), 9: (11, 12), 10: (13, 14), 11: (15,)}.get(
                        ui, ()):
                    emit_part1(eT, i)
        if h == 0:
            for i in range(6, NT):
                ready.append((eT, 0, i))
        if h >= 2:
            # chains two heads back must drain before their eT slot recycles
            while ready and ready[0][1] < h - 1:
                e2, h2, i2 = ready.popleft()
                emit_chain(e2, h2, i2)
    pop_ready(0, force=True)


@functools.lru_cache(maxsize=1)
def _build():
    import concourse.tile as tile
    import concourse.mybir as mybir
    from concourse import bacc
    from contextlib import ExitStack

    f32 = mybir.dt.float32
    bf16 = mybir.dt.bfloat16
    nc = bacc.Bacc(
        "TRN2",
        target_bir_lowering=False,
        debug=False,
        num_devices=N_CORES,
    )
    # q/k/v are pre-cast to bf16 on the host (the kernel computes in bf16
    # anyway), halving input DMA bytes.
    q_d = nc.dram_tensor("q", (T, H, D), bf16, kind="ExternalInput").ap()
    k_d = nc.dram_tensor("k", (T, D), bf16, kind="ExternalInput").ap()
    v_d = nc.dram_tensor("v", (T, D), bf16, kind="ExternalInput").ap()
    o_d = nc.dram_tensor("out", (T, H, D), f32, kind="ExternalOutput").ap()

    with tile.TileContext(nc) as tc:
        with ExitStack() as ctx:
            _build_body(tc, nc, q_d, k_d, v_d, o_d, ctx)
    nc.compile()
    return nc


def _in_maps(q, k, v):
    import ml_dtypes

    bf16 = ml_dtypes.bfloat16
    q = np.asarray(q).astype(bf16)
    k = np.asarray(k).astype(bf16)
    v = np.asarray(v).astype(bf16)
    return [
        {
            "q": np.ascontiguousarray(q[:, H * c:H * c + H, :]),
            "k": np.ascontiguousarray(k[:, c, :]),
            "v": np.ascontiguousarray(v[:, c, :]),
        }
        for c in range(N_CORES)
    ]


def kernel(q, k, v, _trace=False):
    from concourse.bass_utils import run_bass_kernel_spmd

    nc = _build()
    res = run_bass_kernel_spmd(
        nc, _in_maps(q, k, v), core_ids=list(range(N_CORES)), trace=_trace
    )
    out = np.empty((T, H_TOTAL, D), dtype=np.float32)
    for c in range(N_CORES):
        out[:, H * c:H * c + H, :] = res.results[c]["out"].reshape(T, H, D)
    if _trace:
        return out, res
    return out
